# revision 20
# baseline (speedup 1.0000x reference)
"""Soft-KNN Bass/Tile kernel for Trainium2 (8 NeuronCores, axon/PJRT).

Strategy (v3 — single-product f32r, wide selection windows)
-----------------------------------------------------------
- Shard train set (50000 rows) across 8 cores, 6250 rows each, host-side
  sorted by label; a concatenated 800-entry class-boundary table recovers
  labels from global column ids in the final phase only.
- Per core everything is SBUF-resident: x^T as f32r(2x) [128, 4*2048] and
  train^T as f32r(y) [128, 4*6272]; z = f32r(2x)^T.f32r(y) - yn computed
  with 4 f32r matmul passes + 1 bf16 yn-ladder pass per 512-col PSUM chunk
  (f32r = 12-bit-significand fp32: measured end-to-end rel err ~1.5e-2,
  under the 2e-2 gate; no residual cross-terms, no DRAM streaming).
- Selection: z assembled into [128, 2048] windows (4 per qtile);
  vector.max8 + max_index per window -> 32 candidates; exact local top-16
  via max8/match_replace marking + cumsum-rank compaction +
  gpsimd.local_scatter. Candidates ship (value, local col idx) pairs.
- Collective: two AllGather halves ([1024,32] each); core p owns qtiles
  {p, p+8}, so the half-0 global phase overlaps the second half of the
  main loop. Global phase: merge 128 candidates -> top-16, count labels
  against the concatenated boundary table, softmax(-sqrt(xn - z)),
  scatter-add into 100 classes.
- Output per core: [256, 100] = query rows [p*128:+128] and
  [1024+p*128:+128]; host reassembles.
"""

import os
import numpy as np

import concourse.bass as bass
import concourse.bacc as bacc
import concourse.mybir as mybir
import concourse.tile as tile
from concourse import bass_utils
from concourse.masks import make_identity

F32 = mybir.dt.float32
F32R = mybir.dt.float32r
BF16 = mybir.dt.bfloat16
U16 = mybir.dt.uint16
I16 = mybir.dt.int16
I32 = mybir.dt.int32
AL = mybir.AluOpType
AF = mybir.ActivationFunctionType

NCORES = 8
B = 2048                 # queries
D = 512                  # feature dim
NSHARD = 6250            # train rows per core
COLS = 6272              # padded columns (12*512 + 128)
CHUNKS = [512] * 12 + [128]
NCHUNK = len(CHUNKS)     # 13 PSUM chunks
WINDOWS = [(0, 6, 3072), (6, 7, 3200)]  # (first chunk, n chunks, width)
NWIN = len(WINDOWS)
NCAND = 8 * NWIN         # 16 candidates per qtile per core (= what we ship)
QTILES = B // 128        # 16
NCLASS = 100
K = 16
NG = NCORES * K          # 128 gathered candidates
CORESTRIDE = 8192        # global col id = core * CORESTRIDE + local col
NEG = -3.0e38            # match_replace marker
NEGPAD = -1.0e30         # padded-column z value (via yn pad)

STAGE = int(os.environ.get("KNN_STAGE", "3"))


def _merge_top16(nc, small, uniq, vals, width, payloads):
    """Exact top-16 of `vals` [128, width] via max8/match_replace marking +
    cumsum-rank compaction. `payloads`: list of (ap_u16_plane, out_tile) to
    compact with gpsimd.local_scatter in slot order."""
    t8a = small.tile([128, 8], F32, name=f"{uniq}_t8a", tag="mg_t8a")
    t8b = small.tile([128, 8], F32, name=f"{uniq}_t8b", tag="mg_t8b")
    m1 = small.tile([128, NG], F32, name=f"{uniq}_m1", tag="mg_m1")
    m2 = small.tile([128, NG], F32, name=f"{uniq}_m2", tag="mg_m2")
    nc.vector.max(t8a[:], vals[:, :width])
    nc.vector.match_replace(m1[:, :width], t8a[:], vals[:, :width], NEG)
    nc.vector.max(t8b[:], m1[:, :width])
    nc.vector.match_replace(m2[:, :width], t8b[:], m1[:, :width], NEG)
    mask = small.tile([128, NG], F32, name=f"{uniq}_mask", tag="mg_mask")
    nc.vector.tensor_scalar(out=mask[:, :width], in0=m2[:, :width],
                            scalar1=-2e38, scalar2=None, op0=AL.is_le)
    csA = small.tile([128, NG], F32, name=f"{uniq}_csA", tag="mg_csA")
    csB = small.tile([128, NG], F32, name=f"{uniq}_csB", tag="mg_csB")
    nc.vector.tensor_copy(csA[:, :width], mask[:, :width])
    src, dst = csA, csB
    sh = 1
    while sh < width:
        nc.vector.tensor_copy(dst[:, 0:sh], src[:, 0:sh])
        nc.vector.tensor_tensor(out=dst[:, sh:width], in0=src[:, sh:width],
                                in1=src[:, 0:width - sh], op=AL.add)
        src, dst = dst, src
        sh *= 2
    rk = small.tile([128, NG], F32, name=f"{uniq}_rk", tag="mg_rk")
    nc.vector.tensor_tensor(out=rk[:, :width], in0=src[:, :width],
                            in1=mask[:, :width], op=AL.mult)
    nc.vector.tensor_scalar(out=rk[:, :width], in0=rk[:, :width], scalar1=-1.0,
                            scalar2=None, op0=AL.add)
    rk16 = small.tile([128, NG], I16, name=f"{uniq}_rk16", tag="mg_rk16")
    nc.vector.tensor_copy(rk16[:, :width], rk[:, :width])
    for plane, out16 in payloads:
        nc.gpsimd.local_scatter(out16[:].bitcast(I16), plane.bitcast(I16),
                                rk16[:, :width], channels=128, num_elems=K,
                                num_idxs=width)


def build():
    nc = bacc.Bacc("TRN2", target_bir_lowering=False, num_devices=NCORES)

    x_in = nc.dram_tensor("x", [B, D], F32, kind="ExternalInput")
    tr_in = nc.dram_tensor("tr", [NSHARD, D], F32, kind="ExternalInput")
    bnd_in = nc.dram_tensor("bnd", [1, NCORES * NCLASS], F32,
                            kind="ExternalInput")
    out_d = nc.dram_tensor("out", [2 * 128, NCLASS], F32, kind="ExternalOutput")

    yn_bounce = nc.dram_tensor("yn_bounce", [49, 128], F32)
    ag_in = nc.dram_tensor("ag_in", [B, 2 * K], F32)
    ag_out = [nc.dram_tensor(f"ag_out{h}", [NCORES * (B // 2), 2 * K], F32,
                             addr_space="Shared") for h in range(2)]

    with tile.TileContext(nc) as tc:
        with tc.tile_pool(name="res", bufs=1) as res, \
             tc.tile_pool(name="zps", bufs=5, space="PSUM") as zps, \
             tc.tile_pool(name="aux_ps", bufs=2, space="PSUM") as aux_ps:

            # ------------- resident tensors -------------
            ident = res.tile([128, 128], F32)
            make_identity(nc, ident[:])

            base16 = res.tile([128, NCAND], U16)
            nc.gpsimd.iota(base16[:], pattern=[[3072, 2], [0, 8]],
                           channel_multiplier=0)
            coreoff_u = res.tile([128, NG], U16)
            nc.gpsimd.iota(coreoff_u[:], pattern=[[CORESTRIDE, 8], [0, 16]],
                           channel_multiplier=0)
            coreoff_f = res.tile([128, NG], F32)
            nc.vector.tensor_copy(coreoff_f[:], coreoff_u[:])
            coff_row = res.tile([128, NG], U16)
            nc.gpsimd.iota(coff_row[:], pattern=[[NCLASS, 8], [0, 16]],
                           channel_multiplier=0)

            cio_f = res.tile([128, NCLASS], F32)
            bndcat_f = res.tile([128, NCORES * NCLASS], F32)
            ones3 = res.tile([3, 128], BF16)
            nc.vector.memset(ones3[:], -1.0)

            xh_all = res.tile([128, 4 * B], F32R, name="xh_all", tag="xh")
            xn_all = res.tile([128, QTILES], F32)
            trh_all = res.tile([128, 4 * COLS], F32R, name="trh_all", tag="trh")
            yn3 = res.tile([3, COLS], BF16)
            yn_nat = res.tile([128, 49], F32)

            xh_v = xh_all[:].rearrange("p (k n) -> p k n", k=4)
            trh_v = trh_all[:].rearrange("p (k n) -> p k n", k=4)

            # ------------- setup phase -------------
            with tc.tile_pool(name="setup", bufs=2) as sup:
                cio_i = sup.tile([128, NCLASS], I32, tag="cioi", bufs=1)
                nc.gpsimd.iota(cio_i[:], pattern=[[1, NCLASS]],
                               channel_multiplier=0)
                nc.vector.tensor_copy(cio_f[:], cio_i[:])
                bnd_row = sup.tile([1, NCORES * NCLASS], F32, tag="bndrow",
                                   bufs=1)
                nc.sync.dma_start(bnd_row[:], bnd_in[:])
                ones1 = sup.tile([1, 128], F32, tag="ones1", bufs=1)
                nc.vector.memset(ones1[:], 1.0)
                for half, hw in ((0, 512), (512, 288)):
                    bps = aux_ps.tile([128, 512], F32, name=f"bps{half}",
                                      tag="tp")
                    nc.tensor.matmul(bps[:, 0:hw], ones1[:],
                                     bnd_row[0:1, half:half + hw],
                                     start=True, stop=True)
                    nc.scalar.copy(bndcat_f[:, half:half + hw], bps[:, 0:hw])

                # x side: 8 DMAs of 2 qtiles; fused 4-k transpose+cast
                for g in range(8):
                    xt = sup.tile([128, 2 * D], F32, tag="xload")
                    nc.sync.dma_start(
                        xt[:].rearrange("p (j d) -> p j d", d=D),
                        x_in[g * 256:(g + 1) * 256, :]
                        .rearrange("(j p) d -> p j d", p=128))
                    for j in range(2):
                        qt = g * 2 + j
                        junk = sup.tile([128, D], F32, tag="junk")
                        nc.scalar.activation(junk[:], xt[:, j * D:(j + 1) * D],
                                             AF.Square,
                                             accum_out=xn_all[:, qt:qt + 1])
                        tp4 = aux_ps.tile([128, 512], F32, tag="tp")
                        for k in range(4):
                            nc.tensor.transpose(
                                tp4[:, k * 128:(k + 1) * 128],
                                xt[:, j * D + k * 128:j * D + (k + 1) * 128],
                                ident[:])
                        cs = qt * 128
                        dst = xh_v[:, :, cs:cs + 128]
                        src = tp4[:].rearrange("p (k n) -> p k n", k=4)
                        nc.vector.tensor_scalar(out=dst, in0=src,
                                                scalar1=2.0, scalar2=None,
                                                op0=AL.mult)

                # per-chunk yn ladder, interleaved with the train loop so the
                # main loop can start before setup fully drains.
                # rows 1 and 2 of yn3 sit at partitions 1/2, which engine ops
                # cannot address directly; stage through partition 0 + DMA.
                def ladder_chunk(c):
                    cw = CHUNKS[c]
                    co = 512 * c
                    nrow = cw // 128
                    tpc = aux_ps.tile([128, 128], F32, tag="tpy", bufs=1)
                    nc.tensor.transpose(tpc[:nrow, :],
                                        yn_nat[:, 4 * c:4 * c + nrow],
                                        ident[:])
                    stage = sup.tile([4, 128], F32, tag="ynstage")
                    nc.scalar.copy(stage[:nrow, :], tpc[:nrow, :])
                    nc.sync.dma_start(yn_bounce[4 * c:4 * c + nrow, :],
                                      stage[:nrow, :])
                    ynr = sup.tile([1, 512], F32, tag="ynrow")
                    nc.sync.dma_start(
                        ynr[0:1, :cw],
                        yn_bounce[4 * c:4 * c + nrow, :]
                        .rearrange("a b -> (a b)")
                        .rearrange("(o ab) -> o ab", o=1))
                    if c == NCHUNK - 1:
                        nc.vector.memset(ynr[0:1, NSHARD - 6144:cw], -NEGPAD)
                    nc.vector.tensor_copy(yn3[0:1, co:co + cw], ynr[0:1, :cw])
                    nc.vector.tensor_tensor(out=ynr[0:1, :cw],
                                            in0=ynr[0:1, :cw],
                                            in1=yn3[0:1, co:co + cw],
                                            op=AL.subtract)
                    st1 = sup.tile([1, 512], BF16, tag="ystage")
                    nc.vector.tensor_copy(st1[0:1, :cw], ynr[0:1, :cw])
                    nc.sync.dma_start(yn3[1:2, co:co + cw], st1[0:1, :cw])
                    nc.vector.tensor_tensor(out=ynr[0:1, :cw],
                                            in0=ynr[0:1, :cw],
                                            in1=st1[0:1, :cw], op=AL.subtract)
                    st2 = sup.tile([1, 512], BF16, tag="yresid")
                    nc.vector.tensor_copy(st2[0:1, :cw], ynr[0:1, :cw])
                    nc.sync.dma_start(yn3[2:3, co:co + cw], st2[0:1, :cw])

                # train side: 16 DMAs of 3 tiles + tail; fused casts
                nc.vector.memset(yn_nat[:, 48:49], 0.0)
                for k in range(4):
                    nc.vector.memset(
                        trh_all[:, k * COLS + NSHARD:(k + 1) * COLS]
                        .bitcast(F32), 0.0)
                for g in range(17):
                    nt = 3 if g < 16 else 1
                    rows = 128 if g < 16 else 106
                    tt = sup.tile([128, 3 * D], F32, tag="trload")
                    if g < 16:
                        nc.sync.dma_start(
                            tt[:].rearrange("p (j d) -> p j d", d=D),
                            tr_in[g * 384:(g + 1) * 384, :]
                            .rearrange("(j p) d -> p j d", p=128))
                    else:
                        nc.sync.dma_start(tt[:106, 0:D], tr_in[6144:6250, :])
                    for j in range(nt):
                        t = g * 3 + j
                        junk2 = sup.tile([128, D], F32, tag="junk")
                        nc.scalar.activation(junk2[:rows, :],
                                             tt[:rows, j * D:(j + 1) * D],
                                             AF.Square,
                                             accum_out=yn_nat[:rows, t:t + 1])
                        tp4 = aux_ps.tile([128, 512], F32, tag="tp")
                        for k in range(4):
                            nc.tensor.transpose(
                                tp4[:, k * 128:k * 128 + rows],
                                tt[:rows, j * D + k * 128:j * D + (k + 1) * 128],
                                ident[:rows, :rows])
                        cs = t * 128
                        dst = trh_v[:, :, cs:cs + rows]
                        src = tp4[:].rearrange("p (k n) -> p k n", k=4)[:, :, :rows]
                        nc.vector.tensor_copy(dst, src)
                        if t % 4 == 3:
                            ladder_chunk((t - 3) // 4)
                        elif t == 48:
                            ladder_chunk(12)

            # ------------- main + global phase -------------
            with tc.tile_pool(name="wmain", bufs=2) as wmain, \
                 tc.tile_pool(name="candp", bufs=3) as candp, \
                 tc.tile_pool(name="small", bufs=2) as small:

                pid_sp = nc.sync.partition_id()
                HB = B // 2

                def global_phase(l):
                    qrow = pid_sp * 128
                    gv = small.tile([128, NG], F32, name=f"gv{l}", tag="gv")
                    gl = small.tile([128, NG], F32, name=f"gl{l}", tag="gl")
                    for c2 in range(NCORES):
                        nc.sync.dma_start(
                            gv[:, c2 * K:(c2 + 1) * K],
                            ag_out[l][bass.ds(c2 * HB + qrow, 128), 0:K])
                        nc.sync.dma_start(
                            gl[:, c2 * K:(c2 + 1) * K],
                            ag_out[l][bass.ds(c2 * HB + qrow, 128), K:2 * K])
                    nc.vector.tensor_tensor(out=gl[:], in0=gl[:],
                                            in1=coreoff_f[:], op=AL.add)
                    vlo = small.tile([128, NG], U16, name=f"gvlo{l}", tag="vlo")
                    vhi = small.tile([128, NG], U16, name=f"gvhi{l}", tag="vhi")
                    gvu = gv[:].bitcast(U16).rearrange("p (a two) -> p a two",
                                                       two=2)
                    nc.vector.tensor_copy(vlo[:], gvu[:, :, 0:1])
                    nc.vector.tensor_copy(vhi[:], gvu[:, :, 1:2])
                    glu = small.tile([128, NG], U16, name=f"glu{l}", tag="glu")
                    nc.vector.tensor_copy(glu[:], gl[:])
                    slo = small.tile([128, K], U16, name=f"gslo{l}",
                                     tag="slo16")
                    shi = small.tile([128, K], U16, name=f"gshi{l}",
                                     tag="shi16")
                    sla = small.tile([128, K], U16, name=f"gsla{l}",
                                     tag="sgi16")
                    sco = small.tile([128, K], U16, name=f"gsco{l}",
                                     tag="scoff")
                    _merge_top16(nc, small, f"gm{l}", gv, NG,
                                 [(vlo[:], slo), (vhi[:], shi), (glu[:], sla),
                                  (coff_row[:], sco)])
                    v16 = small.tile([128, K], F32, name=f"gv16{l}", tag="v16")
                    v16u = v16[:].bitcast(U16).rearrange(
                        "p (a two) -> p a two", two=2)
                    nc.vector.tensor_copy(v16u[:, :, 0:1], slo[:])
                    nc.vector.tensor_copy(v16u[:, :, 1:2], shi[:])
                    gidx = small.tile([128, K], F32, name=f"gix{l}", tag="gidx")
                    nc.vector.tensor_copy(gidx[:], sla[:])
                    scof = small.tile([128, K], F32, name=f"scf{l}", tag="scof")
                    nc.vector.tensor_copy(scof[:], sco[:])
                    lab16 = small.tile([128, K], F32, name=f"glab{l}",
                                       tag="lab16")
                    cjunk = small.tile([128, NCORES * NCLASS], F32,
                                       name=f"cj{l}", tag="cjunk")
                    for r in range(K):
                        nc.vector.tensor_scalar(
                            out=cjunk[:], in0=bndcat_f[:],
                            scalar1=gidx[:, r:r + 1], scalar2=None,
                            op0=AL.is_le, op1=AL.add,
                            accum_out=lab16[:, r:r + 1])
                    nc.vector.tensor_tensor(out=lab16[:], in0=lab16[:],
                                            in1=scof[:], op=AL.subtract)
                    nc.vector.tensor_scalar(out=lab16[:], in0=lab16[:],
                                            scalar1=-1.0, scalar2=None,
                                            op0=AL.add)
                    xn_col = small.tile([128, 1], F32, name=f"xnc{l}",
                                        tag="xncol")
                    nc.sync.dma_start(xn_col[:],
                                      xn_all[:, bass.ds(pid_sp + 8 * l, 1)])
                    dsq = small.tile([128, K], F32, name=f"dsq{l}", tag="dsq")
                    nc.scalar.activation(dsq[:], v16[:], AF.Sqrt, scale=-1.0,
                                         bias=xn_col[:, 0:1])
                    ew = small.tile([128, K], F32, name=f"ew{l}", tag="ew")
                    zsum = small.tile([128, 1], F32, name=f"zs{l}", tag="zs")
                    nc.scalar.activation(ew[:], dsq[:], AF.Exp, scale=-1.0,
                                         accum_out=zsum[:, 0:1])
                    rz = small.tile([128, 1], F32, name=f"rz{l}", tag="rz")
                    nc.vector.reciprocal(rz[:], zsum[:])
                    wt = small.tile([128, K], F32, name=f"wt{l}", tag="wt")
                    nc.vector.tensor_scalar(out=wt[:], in0=ew[:],
                                            scalar1=rz[:, 0:1], scalar2=None,
                                            op0=AL.mult)
                    vote = small.tile([128, NCLASS], F32, name=f"vote{l}",
                                      tag="vote")
                    tmp = small.tile([128, NCLASS], F32, name=f"vtmp{l}",
                                     tag="vtmp")
                    nc.vector.memset(vote[:], 0.0)
                    for r in range(K):
                        nc.vector.tensor_scalar(out=tmp[:], in0=cio_f[:],
                                                scalar1=lab16[:, r:r + 1],
                                                scalar2=wt[:, r:r + 1],
                                                op0=AL.is_equal, op1=AL.mult)
                        nc.vector.tensor_tensor(out=vote[:], in0=vote[:],
                                                in1=tmp[:], op=AL.add)
                    nc.sync.dma_start(out_d[l * 128:(l + 1) * 128, :], vote[:])

                for qt in range(QTILES if STAGE >= 2 else 0):
                    qs = qt * 128
                    cv = candp.tile([128, NCAND], F32, name=f"cv{qt}", tag="cv")
                    ci = candp.tile([128, NCAND], U16, name=f"ci{qt}", tag="ci")
                    for w, (c0, nsub, ww) in enumerate(WINDOWS):
                        zt = wmain.tile([128, 3200], F32, tag="zt")
                        for sub in range(nsub):
                            c = c0 + sub
                            cw = CHUNKS[c]
                            co = 512 * c
                            ps = zps.tile([128, 512], F32)
                            nc.tensor.matmul(ps[:, :cw], ones3[:],
                                             yn3[:, co:co + cw],
                                             start=True, stop=False)
                            for k in range(4):
                                nc.tensor.matmul(
                                    ps[:, :cw],
                                    xh_v[:, k, qs:qs + 128],
                                    trh_all[:, k * COLS + co:
                                            k * COLS + co + cw],
                                    start=False, stop=(k == 3))
                            nc.scalar.copy(zt[:, sub * 512:sub * 512 + cw],
                                           ps[:, :cw])
                        nc.vector.max(cv[:, w * 8:w * 8 + 8], zt[:, :ww])
                        nc.vector.max_index(ci[:, w * 8:w * 8 + 8],
                                            cv[:, w * 8:w * 8 + 8],
                                            zt[:, :ww])

                    # ship the 16 window-top-8 candidates unsorted; the
                    # global merge sorts by value anyway.
                    gi = small.tile([128, NCAND], U16, name=f"gi{qt}", tag="gi")
                    nc.vector.tensor_tensor(out=gi[:], in0=ci[:],
                                            in1=base16[:], op=AL.add)
                    gf = small.tile([128, K], F32, name=f"gf{qt}", tag="gf")
                    nc.vector.tensor_copy(gf[:], gi[:])
                    nc.sync.dma_start(ag_in[qt * 128:(qt + 1) * 128, 0:K],
                                      cv[:])
                    nc.sync.dma_start(
                        ag_in[qt * 128:(qt + 1) * 128, K:2 * K], gf[:])

                    if STAGE >= 3 and qt in (7, 15):
                        h = qt // 8
                        nc.gpsimd.collective_compute(
                            "AllGather", AL.bypass,
                            replica_groups=[list(range(NCORES))],
                            ins=[ag_in[h * 1024:(h + 1) * 1024, :].opt()],
                            outs=[ag_out[h][:].opt()])
                        global_phase(h)

    nc.finalize()
    return nc


_NC_CACHE = None


def kernel(x, train_features, train_labels, **run_kwargs):
    global _NC_CACHE
    x = np.ascontiguousarray(np.asarray(x, dtype=np.float32))
    tf = np.ascontiguousarray(np.asarray(train_features, dtype=np.float32))
    tl = np.asarray(train_labels)

    bnd_cat = np.zeros(NCORES * NCLASS, np.float32)
    shards = []
    for c in range(NCORES):
        sl = slice(c * NSHARD, (c + 1) * NSHARD)
        labs = np.asarray(tl[sl], dtype=np.int64)
        perm = np.argsort(labs, kind="stable")
        feats_s = np.ascontiguousarray(tf[sl][perm])
        labs_s = labs[perm]
        bnd = np.searchsorted(labs_s, np.arange(NCLASS), side="left")
        bnd_cat[c * NCLASS:(c + 1) * NCLASS] = c * CORESTRIDE + bnd
        shards.append(feats_s)

    in_maps = [{
        "x": x,
        "tr": shards[c],
        "bnd": bnd_cat[None, :],
    } for c in range(NCORES)]

    if _NC_CACHE is None:
        _NC_CACHE = build()
    res = bass_utils.run_bass_kernel_spmd(
        _NC_CACHE, in_maps, core_ids=list(range(NCORES)), **run_kwargs)
    global LAST_RESULTS
    LAST_RESULTS = res
    out = np.zeros((B, NCLASS), np.float32)
    for c in range(NCORES):
        oc = res.results[c]["out"]
        out[c * 128:(c + 1) * 128] = oc[0:128]
        out[1024 + c * 128:1024 + (c + 1) * 128] = oc[128:256]
    return out


LAST_RESULTS = None


# revision 26
# speedup vs baseline: 1.1824x; 1.1824x over previous
"""Soft-KNN Bass/Tile kernel for Trainium2 (8 NeuronCores, axon/PJRT).

Strategy (v3 — single-product f32r, wide selection windows)
-----------------------------------------------------------
- Shard train set (50000 rows) across 8 cores, 6250 rows each, host-side
  sorted by label; a concatenated 800-entry class-boundary table recovers
  labels from global column ids in the final phase only.
- Per core everything is SBUF-resident: x^T as f32r(2x) [128, 4*2048] and
  train^T as f32r(y) [128, 4*6272]; z = f32r(2x)^T.f32r(y) - yn computed
  with 4 f32r matmul passes + 1 bf16 yn-ladder pass per 512-col PSUM chunk
  (f32r = 12-bit-significand fp32: measured end-to-end rel err ~1.5e-2,
  under the 2e-2 gate; no residual cross-terms, no DRAM streaming).
- Selection: z assembled into [128, 2048] windows (4 per qtile);
  vector.max8 + max_index per window -> 32 candidates; exact local top-16
  via max8/match_replace marking + cumsum-rank compaction +
  gpsimd.local_scatter. Candidates ship (value, local col idx) pairs.
- Collective: two AllGather halves ([1024,32] each); core p owns qtiles
  {p, p+8}, so the half-0 global phase overlaps the second half of the
  main loop. Global phase: merge 128 candidates -> top-16, count labels
  against the concatenated boundary table, softmax(-sqrt(xn - z)),
  scatter-add into 100 classes.
- Output per core: [256, 100] = query rows [p*128:+128] and
  [1024+p*128:+128]; host reassembles.
"""

import os
import numpy as np

import concourse.bass as bass
import concourse.bacc as bacc
import concourse.mybir as mybir
import concourse.tile as tile
from concourse import bass_utils
from concourse.masks import make_identity

F32 = mybir.dt.float32
F32R = mybir.dt.float32r
BF16 = mybir.dt.bfloat16
U16 = mybir.dt.uint16
I16 = mybir.dt.int16
I32 = mybir.dt.int32
AL = mybir.AluOpType
AF = mybir.ActivationFunctionType

NCORES = 8
B = 2048                 # queries
D = 512                  # feature dim
NSHARD = 6250            # train rows per core
COLS = 6400              # padded columns (12*512 + 256)
CHUNKS = [512] * 12 + [256]
NCHUNK = len(CHUNKS)     # 13 PSUM chunks
WINDOWS = [(0, 6, 3072), (6, 7, 3328)]  # (first chunk, n chunks, width)
NWIN = len(WINDOWS)
NCAND = 8 * NWIN         # 16 candidates per qtile per core (= what we ship)
QTILES = B // 128        # 16
NCLASS = 100
K = 16
NG = NCORES * K          # 128 gathered candidates
CORESTRIDE = 8192        # global col id = core * CORESTRIDE + local col
NEG = -3.0e38            # match_replace marker
NEGPAD = -1.0e30         # padded-column z value (via yn pad)

STAGE = int(os.environ.get("KNN_STAGE", "3"))


def _merge_top16(nc, small, uniq, vals, width, payloads):
    """Exact top-16 of `vals` [128, width] via max8/match_replace marking +
    cumsum-rank compaction. `payloads`: list of (ap_u16_plane, out_tile) to
    compact with gpsimd.local_scatter in slot order."""
    t8a = small.tile([128, 8], F32, name=f"{uniq}_t8a", tag="mg_t8a")
    t8b = small.tile([128, 8], F32, name=f"{uniq}_t8b", tag="mg_t8b")
    m1 = small.tile([128, NG], F32, name=f"{uniq}_m1", tag="mg_m1")
    m2 = small.tile([128, NG], F32, name=f"{uniq}_m2", tag="mg_m2")
    nc.vector.max(t8a[:], vals[:, :width])
    nc.vector.match_replace(m1[:, :width], t8a[:], vals[:, :width], NEG)
    nc.vector.max(t8b[:], m1[:, :width])
    nc.vector.match_replace(m2[:, :width], t8b[:], m1[:, :width], NEG)
    mask = small.tile([128, NG], F32, name=f"{uniq}_mask", tag="mg_mask")
    nc.vector.tensor_scalar(out=mask[:, :width], in0=m2[:, :width],
                            scalar1=-2e38, scalar2=None, op0=AL.is_le)
    csA = small.tile([128, NG], F32, name=f"{uniq}_csA", tag="mg_csA")
    csB = small.tile([128, NG], F32, name=f"{uniq}_csB", tag="mg_csB")
    nc.vector.tensor_copy(csA[:, :width], mask[:, :width])
    src, dst = csA, csB
    sh = 1
    while sh < width:
        nc.vector.tensor_copy(dst[:, 0:sh], src[:, 0:sh])
        nc.vector.tensor_tensor(out=dst[:, sh:width], in0=src[:, sh:width],
                                in1=src[:, 0:width - sh], op=AL.add)
        src, dst = dst, src
        sh *= 2
    rk = small.tile([128, NG], F32, name=f"{uniq}_rk", tag="mg_rk")
    nc.vector.tensor_tensor(out=rk[:, :width], in0=src[:, :width],
                            in1=mask[:, :width], op=AL.mult)
    nc.vector.tensor_scalar(out=rk[:, :width], in0=rk[:, :width], scalar1=-1.0,
                            scalar2=None, op0=AL.add)
    rk16 = small.tile([128, NG], I16, name=f"{uniq}_rk16", tag="mg_rk16")
    nc.vector.tensor_copy(rk16[:, :width], rk[:, :width])
    for plane, out16 in payloads:
        nc.gpsimd.local_scatter(out16[:].bitcast(I16), plane.bitcast(I16),
                                rk16[:, :width], channels=128, num_elems=K,
                                num_idxs=width)


def build():
    nc = bacc.Bacc("TRN2", target_bir_lowering=False, num_devices=NCORES)

    x_in = nc.dram_tensor("x", [B, D], F32, kind="ExternalInput")
    tr_in = nc.dram_tensor("tr", [NSHARD, D], F32, kind="ExternalInput")
    bnd_in = nc.dram_tensor("bnd", [1, NCORES * NCLASS], F32,
                            kind="ExternalInput")
    out_d = nc.dram_tensor("out", [2 * 128, NCLASS], F32, kind="ExternalOutput")

    yn_bounce = nc.dram_tensor("yn_bounce", [50, 128], F32)
    ag_in = nc.dram_tensor("ag_in", [B, 2 * K], F32)
    ag_out = [nc.dram_tensor(f"ag_out{h}", [B // 2, 2 * K], F32)
              for h in range(2)]

    with tile.TileContext(nc) as tc:
        with tc.tile_pool(name="res", bufs=1) as res, \
             tc.tile_pool(name="zps", bufs=5, space="PSUM") as zps, \
             tc.tile_pool(name="aux_ps", bufs=2, space="PSUM") as aux_ps:

            # ------------- resident tensors -------------
            ident = res.tile([128, 128], F32)
            make_identity(nc, ident[:])

            base16 = res.tile([128, NCAND], U16)
            nc.gpsimd.iota(base16[:], pattern=[[3072, 2], [0, 8]],
                           channel_multiplier=0)
            coreoff_u = res.tile([128, NG], U16)
            nc.gpsimd.iota(coreoff_u[:], pattern=[[CORESTRIDE, 8], [0, 16]],
                           channel_multiplier=0)
            coreoff_f = res.tile([128, NG], F32)
            nc.vector.tensor_copy(coreoff_f[:], coreoff_u[:])
            coff_row = res.tile([128, NG], U16)
            nc.gpsimd.iota(coff_row[:], pattern=[[NCLASS, 8], [0, 16]],
                           channel_multiplier=0)

            cio_f = res.tile([128, NCLASS], F32)
            bndcat_f = res.tile([128, NCORES * NCLASS], F32)
            bndcat_u = res.tile([128, NCORES * NCLASS], U16)
            ones3 = res.tile([3, 128], BF16)
            nc.vector.memset(ones3[:], -1.0)

            xh_all = res.tile([128, 4 * B], F32R, name="xh_all", tag="xh")
            xn_all = res.tile([128, QTILES], F32)
            trh_all = res.tile([128, 4 * COLS], F32R, name="trh_all", tag="trh")
            yn3 = res.tile([3, COLS], BF16)
            yn_nat = res.tile([128, 50], F32)

            xh_v = xh_all[:].rearrange("p (k n) -> p k n", k=4)
            trh_v = trh_all[:].rearrange("p (k n) -> p k n", k=4)

            # ------------- setup phase -------------
            with tc.tile_pool(name="setup", bufs=2) as sup:
                cio_i = sup.tile([128, NCLASS], I32, tag="cioi", bufs=1)
                nc.gpsimd.iota(cio_i[:], pattern=[[1, NCLASS]],
                               channel_multiplier=0)
                nc.vector.tensor_copy(cio_f[:], cio_i[:])
                bnd_row = sup.tile([1, NCORES * NCLASS], F32, tag="bndrow",
                                   bufs=1)
                nc.sync.dma_start(bnd_row[:], bnd_in[:])
                ones1 = sup.tile([1, 128], F32, tag="ones1", bufs=1)
                nc.vector.memset(ones1[:], 1.0)
                for half, hw in ((0, 512), (512, 288)):
                    bps = aux_ps.tile([128, 512], F32, name=f"bps{half}",
                                      tag="tp")
                    nc.tensor.matmul(bps[:, 0:hw], ones1[:],
                                     bnd_row[0:1, half:half + hw],
                                     start=True, stop=True)
                    nc.scalar.copy(bndcat_f[:, half:half + hw], bps[:, 0:hw])
                nc.vector.tensor_copy(bndcat_u[:], bndcat_f[:])

                # x side: 8 DMAs of 2 qtiles; fused 4-k transpose+cast.
                # group 0 is emitted first (main qtile 0 needs it); groups
                # 1-7 interleave with the train loop below.
                def x_group(g):
                    xt = sup.tile([128, 2 * D], F32, tag="xload")
                    nc.sync.dma_start(
                        xt[:].rearrange("p (j d) -> p j d", d=D),
                        x_in[g * 256:(g + 1) * 256, :]
                        .rearrange("(j p) d -> p j d", p=128))
                    for j in range(2):
                        qt = g * 2 + j
                        junk = sup.tile([128, D], F32, tag="junk")
                        nc.scalar.activation(junk[:], xt[:, j * D:(j + 1) * D],
                                             AF.Square,
                                             accum_out=xn_all[:, qt:qt + 1])
                        tp4 = aux_ps.tile([128, 512], F32, tag="tp")
                        for k in range(4):
                            nc.tensor.transpose(
                                tp4[:, k * 128:(k + 1) * 128],
                                xt[:, j * D + k * 128:j * D + (k + 1) * 128],
                                ident[:])
                        cs = qt * 128
                        dst = xh_v[:, :, cs:cs + 128]
                        src = tp4[:].rearrange("p (k n) -> p k n", k=4)
                        nc.vector.tensor_scalar(out=dst, in0=src,
                                                scalar1=2.0, scalar2=None,
                                                op0=AL.mult)

                x_group(0)

                # per-chunk yn ladder, interleaved with the train loop so the
                # main loop can start before setup fully drains.
                # rows 1 and 2 of yn3 sit at partitions 1/2, which engine ops
                # cannot address directly; stage through partition 0 + DMA.
                def ladder_chunk(c):
                    cw = CHUNKS[c]
                    co = 512 * c
                    nrow = cw // 128
                    tpc = aux_ps.tile([128, 128], F32, tag="tpy", bufs=1)
                    nc.tensor.transpose(tpc[:nrow, :],
                                        yn_nat[:, 4 * c:4 * c + nrow],
                                        ident[:])
                    stage = sup.tile([4, 128], F32, tag="ynstage")
                    nc.scalar.copy(stage[:nrow, :], tpc[:nrow, :])
                    nc.sync.dma_start(yn_bounce[4 * c:4 * c + nrow, :],
                                      stage[:nrow, :])
                    ynr = sup.tile([1, 512], F32, tag="ynrow")
                    nc.sync.dma_start(
                        ynr[0:1, :cw],
                        yn_bounce[4 * c:4 * c + nrow, :]
                        .rearrange("a b -> (a b)")
                        .rearrange("(o ab) -> o ab", o=1))
                    if c == NCHUNK - 1:
                        nc.vector.memset(ynr[0:1, NSHARD - 6144:cw], -NEGPAD)
                    nc.vector.tensor_copy(yn3[0:1, co:co + cw], ynr[0:1, :cw])
                    nc.vector.tensor_tensor(out=ynr[0:1, :cw],
                                            in0=ynr[0:1, :cw],
                                            in1=yn3[0:1, co:co + cw],
                                            op=AL.subtract)
                    st1 = sup.tile([1, 512], BF16, tag="ystage")
                    nc.vector.tensor_copy(st1[0:1, :cw], ynr[0:1, :cw])
                    nc.sync.dma_start(yn3[1:2, co:co + cw], st1[0:1, :cw])
                    nc.vector.tensor_tensor(out=ynr[0:1, :cw],
                                            in0=ynr[0:1, :cw],
                                            in1=st1[0:1, :cw], op=AL.subtract)
                    st2 = sup.tile([1, 512], BF16, tag="yresid")
                    nc.vector.tensor_copy(st2[0:1, :cw], ynr[0:1, :cw])
                    nc.sync.dma_start(yn3[2:3, co:co + cw], st2[0:1, :cw])

                # train side: 16 DMAs of 3 tiles + tail; fused casts
                nc.vector.memset(yn_nat[:, 48:50], 0.0)
                for k in range(4):
                    nc.vector.memset(
                        trh_all[:, k * COLS + NSHARD:(k + 1) * COLS]
                        .bitcast(F32), 0.0)
                for g in range(17):
                    nt = 3 if g < 16 else 1
                    rows = 128 if g < 16 else 106
                    tt = sup.tile([128, 3 * D], F32, tag="trload")
                    if g < 16:
                        nc.sync.dma_start(
                            tt[:].rearrange("p (j d) -> p j d", d=D),
                            tr_in[g * 384:(g + 1) * 384, :]
                            .rearrange("(j p) d -> p j d", p=128))
                    else:
                        nc.sync.dma_start(tt[:106, 0:D], tr_in[6144:6250, :])
                    for j in range(nt):
                        t = g * 3 + j
                        junk2 = sup.tile([128, D], F32, tag="junk")
                        nc.scalar.activation(junk2[:rows, :],
                                             tt[:rows, j * D:(j + 1) * D],
                                             AF.Square,
                                             accum_out=yn_nat[:rows, t:t + 1])
                        tp4 = aux_ps.tile([128, 512], F32, tag="tp")
                        for k in range(4):
                            nc.tensor.transpose(
                                tp4[:, k * 128:k * 128 + rows],
                                tt[:rows, j * D + k * 128:j * D + (k + 1) * 128],
                                ident[:rows, :rows])
                        cs = t * 128
                        dst = trh_v[:, :, cs:cs + rows]
                        src = tp4[:].rearrange("p (k n) -> p k n", k=4)[:, :, :rows]
                        nc.vector.tensor_copy(dst, src)
                        if t % 4 == 3:
                            ladder_chunk((t - 3) // 4)
                        elif t == 48:
                            ladder_chunk(12)
                    if g % 2 == 1 and g // 2 + 1 <= 7:
                        x_group(g // 2 + 1)

            # ------------- main + global phase -------------
            with tc.tile_pool(name="wmain", bufs=2) as wmain, \
                 tc.tile_pool(name="candp", bufs=3) as candp, \
                 tc.tile_pool(name="small", bufs=2) as small:

                pid_sp = nc.sync.partition_id()
                HB = B // 2

                def global_phase(l):
                    gv = small.tile([128, NG], F32, name=f"gv{l}", tag="gv")
                    gl = small.tile([128, NG], F32, name=f"gl{l}", tag="gl")
                    src = ag_out[l][:].rearrange("(c r) f -> r c f", c=NCORES)
                    nc.sync.dma_start(
                        gv[:].rearrange("p (c k) -> p c k", c=NCORES),
                        src[:, :, 0:K])
                    nc.sync.dma_start(
                        gl[:].rearrange("p (c k) -> p c k", c=NCORES),
                        src[:, :, K:2 * K])
                    nc.vector.tensor_tensor(out=gl[:], in0=gl[:],
                                            in1=coreoff_f[:], op=AL.add)
                    vlo = small.tile([128, NG], U16, name=f"gvlo{l}", tag="vlo")
                    vhi = small.tile([128, NG], U16, name=f"gvhi{l}", tag="vhi")
                    gvu = gv[:].bitcast(U16).rearrange("p (a two) -> p a two",
                                                       two=2)
                    nc.vector.tensor_copy(vlo[:], gvu[:, :, 0:1])
                    nc.vector.tensor_copy(vhi[:], gvu[:, :, 1:2])
                    glu = small.tile([128, NG], U16, name=f"glu{l}", tag="glu")
                    nc.vector.tensor_copy(glu[:], gl[:])
                    slo = small.tile([128, K], U16, name=f"gslo{l}",
                                     tag="slo16")
                    shi = small.tile([128, K], U16, name=f"gshi{l}",
                                     tag="shi16")
                    sla = small.tile([128, K], U16, name=f"gsla{l}",
                                     tag="sgi16")
                    sco = small.tile([128, K], U16, name=f"gsco{l}",
                                     tag="scoff")
                    _merge_top16(nc, small, f"gm{l}", gv, NG,
                                 [(vlo[:], slo), (vhi[:], shi), (glu[:], sla),
                                  (coff_row[:], sco)])
                    v16 = small.tile([128, K], F32, name=f"gv16{l}", tag="v16")
                    v16u = v16[:].bitcast(U16).rearrange(
                        "p (a two) -> p a two", two=2)
                    nc.vector.tensor_copy(v16u[:, :, 0:1], slo[:])
                    nc.vector.tensor_copy(v16u[:, :, 1:2], shi[:])
                    scof = small.tile([128, K], F32, name=f"scf{l}", tag="scof")
                    nc.vector.tensor_copy(scof[:], sco[:])
                    gidx = small.tile([128, K], F32, name=f"gix{l}",
                                      tag="gidx")
                    nc.vector.tensor_copy(gidx[:], sla[:])
                    cjunk = small.tile([128, NCORES * NCLASS], U16,
                                       name=f"cj{l}", tag="cjunk")
                    lab16 = small.tile([128, K], F32, name=f"glab{l}",
                                       tag="lab16")
                    for r in range(K):
                        nc.vector.tensor_scalar(
                            out=cjunk[:], in0=bndcat_u[:],
                            scalar1=gidx[:, r:r + 1], scalar2=None,
                            op0=AL.is_le, op1=AL.add,
                            accum_out=lab16[:, r:r + 1])
                    nc.vector.tensor_tensor(out=lab16[:], in0=lab16[:],
                                            in1=scof[:], op=AL.subtract)
                    nc.vector.tensor_scalar(out=lab16[:], in0=lab16[:],
                                            scalar1=-1.0, scalar2=None,
                                            op0=AL.add)
                    xn_col = small.tile([128, 1], F32, name=f"xnc{l}",
                                        tag="xncol")
                    nc.sync.dma_start(xn_col[:],
                                      xn_all[:, bass.ds(pid_sp + 8 * l, 1)])
                    dsq = small.tile([128, K], F32, name=f"dsq{l}", tag="dsq")
                    nc.scalar.activation(dsq[:], v16[:], AF.Sqrt, scale=-1.0,
                                         bias=xn_col[:, 0:1])
                    ew = small.tile([128, K], F32, name=f"ew{l}", tag="ew")
                    zsum = small.tile([128, 1], F32, name=f"zs{l}", tag="zs")
                    nc.scalar.activation(ew[:], dsq[:], AF.Exp, scale=-1.0,
                                         accum_out=zsum[:, 0:1])
                    rz = small.tile([128, 1], F32, name=f"rz{l}", tag="rz")
                    nc.vector.reciprocal(rz[:], zsum[:])
                    wt = small.tile([128, K], F32, name=f"wt{l}", tag="wt")
                    nc.vector.tensor_scalar(out=wt[:], in0=ew[:],
                                            scalar1=rz[:, 0:1], scalar2=None,
                                            op0=AL.mult)
                    vote = small.tile([128, NCLASS], F32, name=f"vote{l}",
                                      tag="vote")
                    tmp = small.tile([128, NCLASS], F32, name=f"vtmp{l}",
                                     tag="vtmp")
                    nc.vector.memset(vote[:], 0.0)
                    for r in range(K):
                        nc.vector.tensor_scalar(out=tmp[:], in0=cio_f[:],
                                                scalar1=lab16[:, r:r + 1],
                                                scalar2=wt[:, r:r + 1],
                                                op0=AL.is_equal, op1=AL.mult)
                        nc.vector.tensor_tensor(out=vote[:], in0=vote[:],
                                                in1=tmp[:], op=AL.add)
                    nc.sync.dma_start(out_d[l * 128:(l + 1) * 128, :], vote[:])

                for qt in range(QTILES if STAGE >= 2 else 0):
                    qs = qt * 128
                    cv = candp.tile([128, NCAND], F32, name=f"cv{qt}", tag="cv")
                    ci = candp.tile([128, NCAND], U16, name=f"ci{qt}", tag="ci")
                    for w, (c0, nsub, ww) in enumerate(WINDOWS):
                        zt = wmain.tile([128, 3328], F32, tag="zt")
                        for sub in range(nsub):
                            c = c0 + sub
                            cw = CHUNKS[c]
                            co = 512 * c
                            ps = zps.tile([128, 512], F32)
                            nc.tensor.matmul(ps[:, :cw], ones3[:],
                                             yn3[:, co:co + cw],
                                             start=True, stop=False)
                            for k in range(4):
                                nc.tensor.matmul(
                                    ps[:, :cw],
                                    xh_v[:, k, qs:qs + 128],
                                    trh_all[:, k * COLS + co:
                                            k * COLS + co + cw],
                                    start=False, stop=(k == 3))
                            nc.scalar.copy(zt[:, sub * 512:sub * 512 + cw],
                                           ps[:, :cw])
                        nc.vector.max(cv[:, w * 8:w * 8 + 8], zt[:, :ww])
                        nc.vector.max_index(ci[:, w * 8:w * 8 + 8],
                                            cv[:, w * 8:w * 8 + 8],
                                            zt[:, :ww])

                    # ship the 16 window-top-8 candidates unsorted; the
                    # global merge sorts by value anyway.
                    gi = small.tile([128, NCAND], U16, name=f"gi{qt}", tag="gi")
                    nc.vector.tensor_tensor(out=gi[:], in0=ci[:],
                                            in1=base16[:], op=AL.add)
                    gf = small.tile([128, K], F32, name=f"gf{qt}", tag="gf")
                    nc.vector.tensor_copy(gf[:], gi[:])
                    nc.sync.dma_start(ag_in[qt * 128:(qt + 1) * 128, 0:K],
                                      cv[:])
                    nc.sync.dma_start(
                        ag_in[qt * 128:(qt + 1) * 128, K:2 * K], gf[:])

                    if STAGE >= 3 and qt in (7, 15):
                        h = qt // 8
                        nc.gpsimd.collective_compute(
                            "AllToAll", AL.bypass,
                            replica_groups=[list(range(NCORES))],
                            ins=[ag_in[h * 1024:(h + 1) * 1024, :].opt()],
                            outs=[ag_out[h][:].opt()])
                        global_phase(h)

    nc.finalize()
    return nc


_NC_CACHE = None


def kernel(x, train_features, train_labels, **run_kwargs):
    global _NC_CACHE
    x = np.ascontiguousarray(np.asarray(x, dtype=np.float32))
    tf = np.ascontiguousarray(np.asarray(train_features, dtype=np.float32))
    tl = np.asarray(train_labels)

    bnd_cat = np.zeros(NCORES * NCLASS, np.float32)
    shards = []
    for c in range(NCORES):
        sl = slice(c * NSHARD, (c + 1) * NSHARD)
        labs = np.asarray(tl[sl], dtype=np.int64)
        perm = np.argsort(labs, kind="stable")
        feats_s = np.ascontiguousarray(tf[sl][perm])
        labs_s = labs[perm]
        bnd = np.searchsorted(labs_s, np.arange(NCLASS), side="left")
        bnd_cat[c * NCLASS:(c + 1) * NCLASS] = c * CORESTRIDE + bnd
        shards.append(feats_s)

    in_maps = [{
        "x": x,
        "tr": shards[c],
        "bnd": bnd_cat[None, :],
    } for c in range(NCORES)]

    if _NC_CACHE is None:
        _NC_CACHE = build()
    res = bass_utils.run_bass_kernel_spmd(
        _NC_CACHE, in_maps, core_ids=list(range(NCORES)), **run_kwargs)
    global LAST_RESULTS
    LAST_RESULTS = res
    out = np.zeros((B, NCLASS), np.float32)
    for c in range(NCORES):
        oc = res.results[c]["out"]
        out[c * 128:(c + 1) * 128] = oc[0:128]
        out[1024 + c * 128:1024 + (c + 1) * 128] = oc[128:256]
    return out


LAST_RESULTS = None


# revision 28
# speedup vs baseline: 1.2270x; 1.0377x over previous
"""Soft-KNN Bass/Tile kernel for Trainium2 (8 NeuronCores, axon/PJRT).

Strategy (v3 — single-product f32r, wide selection windows)
-----------------------------------------------------------
- Shard train set (50000 rows) across 8 cores, 6250 rows each, host-side
  sorted by label; a concatenated 800-entry class-boundary table recovers
  labels from global column ids in the final phase only.
- Per core everything is SBUF-resident: x^T as f32r(2x) [128, 4*2048] and
  train^T as f32r(y) [128, 4*6272]; z = f32r(2x)^T.f32r(y) - yn computed
  with 4 f32r matmul passes + 1 bf16 yn-ladder pass per 512-col PSUM chunk
  (f32r = 12-bit-significand fp32: measured end-to-end rel err ~1.5e-2,
  under the 2e-2 gate; no residual cross-terms, no DRAM streaming).
- Selection: z assembled into [128, 2048] windows (4 per qtile);
  vector.max8 + max_index per window -> 32 candidates; exact local top-16
  via max8/match_replace marking + cumsum-rank compaction +
  gpsimd.local_scatter. Candidates ship (value, local col idx) pairs.
- Collective: two AllGather halves ([1024,32] each); core p owns qtiles
  {p, p+8}, so the half-0 global phase overlaps the second half of the
  main loop. Global phase: merge 128 candidates -> top-16, count labels
  against the concatenated boundary table, softmax(-sqrt(xn - z)),
  scatter-add into 100 classes.
- Output per core: [256, 100] = query rows [p*128:+128] and
  [1024+p*128:+128]; host reassembles.
"""

import os
import numpy as np

import concourse.bass as bass
import concourse.bacc as bacc
import concourse.mybir as mybir
import concourse.tile as tile
from concourse import bass_utils
from concourse.masks import make_identity

F32 = mybir.dt.float32
F32R = mybir.dt.float32r
BF16 = mybir.dt.bfloat16
U16 = mybir.dt.uint16
I16 = mybir.dt.int16
I32 = mybir.dt.int32
AL = mybir.AluOpType
AF = mybir.ActivationFunctionType

NCORES = 8
B = 2048                 # queries
D = 512                  # feature dim
NSHARD = 6250            # train rows per core
COLS = 6400              # padded columns (12*512 + 256)
CHUNKS = [512] * 12 + [256]
NCHUNK = len(CHUNKS)     # 13 PSUM chunks
WINDOWS = [(0, 6, 3072), (6, 7, 3328)]  # (first chunk, n chunks, width)
NWIN = len(WINDOWS)
NCAND = 8 * NWIN         # 16 candidates per qtile per core (= what we ship)
QTILES = B // 128        # 16
NCLASS = 100
K = 16
NG = NCORES * K          # 128 gathered candidates
CORESTRIDE = 8192        # global col id = core * CORESTRIDE + local col
NEG = -3.0e38            # match_replace marker
NEGPAD = -1.0e30         # padded-column z value (via yn pad)

STAGE = int(os.environ.get("KNN_STAGE", "3"))


def _merge_top16(nc, small, uniq, vals, width, payloads):
    """Exact top-16 of `vals` [128, width] via max8/match_replace marking +
    cumsum-rank compaction. `payloads`: list of (ap_u16_plane, out_tile) to
    compact with gpsimd.local_scatter in slot order."""
    t8a = small.tile([128, 8], F32, name=f"{uniq}_t8a", tag="mg_t8a")
    t8b = small.tile([128, 8], F32, name=f"{uniq}_t8b", tag="mg_t8b")
    m1 = small.tile([128, NG], F32, name=f"{uniq}_m1", tag="mg_m1")
    m2 = small.tile([128, NG], F32, name=f"{uniq}_m2", tag="mg_m2")
    nc.vector.max(t8a[:], vals[:, :width])
    nc.vector.match_replace(m1[:, :width], t8a[:], vals[:, :width], NEG)
    nc.vector.max(t8b[:], m1[:, :width])
    nc.vector.match_replace(m2[:, :width], t8b[:], m1[:, :width], NEG)
    mask = small.tile([128, NG], F32, name=f"{uniq}_mask", tag="mg_mask")
    nc.vector.tensor_scalar(out=mask[:, :width], in0=m2[:, :width],
                            scalar1=-2e38, scalar2=None, op0=AL.is_le)
    csA = small.tile([128, NG], F32, name=f"{uniq}_csA", tag="mg_csA")
    csB = small.tile([128, NG], F32, name=f"{uniq}_csB", tag="mg_csB")
    nc.vector.tensor_copy(csA[:, :width], mask[:, :width])
    src, dst = csA, csB
    sh = 1
    while sh < width:
        nc.vector.tensor_copy(dst[:, 0:sh], src[:, 0:sh])
        nc.vector.tensor_tensor(out=dst[:, sh:width], in0=src[:, sh:width],
                                in1=src[:, 0:width - sh], op=AL.add)
        src, dst = dst, src
        sh *= 2
    rk = small.tile([128, NG], F32, name=f"{uniq}_rk", tag="mg_rk")
    nc.vector.tensor_tensor(out=rk[:, :width], in0=src[:, :width],
                            in1=mask[:, :width], op=AL.mult)
    nc.vector.tensor_scalar(out=rk[:, :width], in0=rk[:, :width], scalar1=-1.0,
                            scalar2=None, op0=AL.add)
    rk16 = small.tile([128, NG], I16, name=f"{uniq}_rk16", tag="mg_rk16")
    nc.vector.tensor_copy(rk16[:, :width], rk[:, :width])
    for plane, out16 in payloads:
        nc.gpsimd.local_scatter(out16[:].bitcast(I16), plane.bitcast(I16),
                                rk16[:, :width], channels=128, num_elems=K,
                                num_idxs=width)


def build():
    nc = bacc.Bacc("TRN2", target_bir_lowering=False, num_devices=NCORES)

    x_in = nc.dram_tensor("x", [B, D], F32, kind="ExternalInput")
    tr_in = nc.dram_tensor("tr", [NSHARD, D], F32, kind="ExternalInput")
    bnd_in = nc.dram_tensor("bnd", [1, NCORES * NCLASS], F32,
                            kind="ExternalInput")
    out_d = nc.dram_tensor("out", [2 * 128, NCLASS], F32, kind="ExternalOutput")

    yn_bounce = nc.dram_tensor("yn_bounce", [50, 128], F32)
    ag_in = nc.dram_tensor("ag_in", [B, 2 * K], F32)
    ag_out = [nc.dram_tensor(f"ag_out{h}", [B // 2, 2 * K], F32)
              for h in range(2)]

    with tile.TileContext(nc) as tc:
        with tc.tile_pool(name="res", bufs=1) as res, \
             tc.tile_pool(name="zps", bufs=5, space="PSUM") as zps, \
             tc.tile_pool(name="aux_ps", bufs=2, space="PSUM") as aux_ps:

            # ------------- resident tensors -------------
            ident = res.tile([128, 128], F32)
            make_identity(nc, ident[:])

            base16 = res.tile([128, NCAND], U16)
            nc.gpsimd.iota(base16[:], pattern=[[3072, 2], [0, 8]],
                           channel_multiplier=0)
            coreoff_u = res.tile([128, NG], U16)
            nc.gpsimd.iota(coreoff_u[:], pattern=[[CORESTRIDE, 8], [0, 16]],
                           channel_multiplier=0)
            coreoff_f = res.tile([128, NG], F32)
            nc.vector.tensor_copy(coreoff_f[:], coreoff_u[:])
            coff_row = res.tile([128, NG], U16)
            nc.gpsimd.iota(coff_row[:], pattern=[[NCLASS, 8], [0, 16]],
                           channel_multiplier=0)

            cio_f = res.tile([128, NCLASS], F32)
            bndcat_f = res.tile([128, NCORES * NCLASS], F32)
            bndcat_u = res.tile([128, NCORES * NCLASS], U16)
            ones3 = res.tile([3, 128], BF16)
            nc.vector.memset(ones3[:], -1.0)

            xh_all = res.tile([128, 4 * B], F32R, name="xh_all", tag="xh")
            xn_all = res.tile([128, QTILES], F32)
            trh_all = res.tile([128, 4 * COLS], F32R, name="trh_all", tag="trh")
            yn3 = res.tile([3, COLS], BF16)
            yn_nat = res.tile([128, 50], F32)

            xh_v = xh_all[:].rearrange("p (k n) -> p k n", k=4)
            trh_v = trh_all[:].rearrange("p (k n) -> p k n", k=4)

            # ------------- setup phase -------------
            with tc.tile_pool(name="setup", bufs=2) as sup:
                cio_i = sup.tile([128, NCLASS], I32, tag="cioi", bufs=1)
                nc.gpsimd.iota(cio_i[:], pattern=[[1, NCLASS]],
                               channel_multiplier=0)
                nc.vector.tensor_copy(cio_f[:], cio_i[:])
                bnd_row = sup.tile([1, NCORES * NCLASS], F32, tag="bndrow",
                                   bufs=1)
                nc.sync.dma_start(bnd_row[:], bnd_in[:])
                ones1 = sup.tile([1, 128], F32, tag="ones1", bufs=1)
                nc.vector.memset(ones1[:], 1.0)
                for half, hw in ((0, 512), (512, 288)):
                    bps = aux_ps.tile([128, 512], F32, name=f"bps{half}",
                                      tag="tp")
                    nc.tensor.matmul(bps[:, 0:hw], ones1[:],
                                     bnd_row[0:1, half:half + hw],
                                     start=True, stop=True)
                    nc.scalar.copy(bndcat_f[:, half:half + hw], bps[:, 0:hw])
                nc.vector.tensor_copy(bndcat_u[:], bndcat_f[:])

                # x side: 8 DMAs of 2 qtiles; fused 4-k transpose+cast.
                # group 0 is emitted first (main qtile 0 needs it); groups
                # 1-7 interleave with the train loop below.
                def x_group(g):
                    xt = sup.tile([128, 2 * D], F32, tag="xload")
                    nc.sync.dma_start(
                        xt[:].rearrange("p (j d) -> p j d", d=D),
                        x_in[g * 256:(g + 1) * 256, :]
                        .rearrange("(j p) d -> p j d", p=128))
                    for j in range(2):
                        qt = g * 2 + j
                        junk = sup.tile([128, D], F32, tag="junk")
                        nc.scalar.activation(junk[:], xt[:, j * D:(j + 1) * D],
                                             AF.Square,
                                             accum_out=xn_all[:, qt:qt + 1])
                        tp4 = aux_ps.tile([128, 512], F32, tag="tp")
                        for k in range(4):
                            nc.tensor.transpose(
                                tp4[:, k * 128:(k + 1) * 128],
                                xt[:, j * D + k * 128:j * D + (k + 1) * 128],
                                ident[:])
                        cs = qt * 128
                        dst = xh_v[:, :, cs:cs + 128]
                        src = tp4[:].rearrange("p (k n) -> p k n", k=4)
                        nc.vector.tensor_scalar(out=dst, in0=src,
                                                scalar1=2.0, scalar2=None,
                                                op0=AL.mult)

                x_group(0)

                # per-chunk yn ladder, interleaved with the train loop so the
                # main loop can start before setup fully drains.
                # rows 1 and 2 of yn3 sit at partitions 1/2, which engine ops
                # cannot address directly; stage through partition 0 + DMA.
                def ladder_chunk(c):
                    cw = CHUNKS[c]
                    co = 512 * c
                    nrow = cw // 128
                    tpc = aux_ps.tile([128, 128], F32, tag="tpy", bufs=1)
                    nc.tensor.transpose(tpc[:nrow, :],
                                        yn_nat[:, 4 * c:4 * c + nrow],
                                        ident[:])
                    stage = sup.tile([4, 128], F32, tag="ynstage")
                    nc.scalar.copy(stage[:nrow, :], tpc[:nrow, :])
                    nc.sync.dma_start(yn_bounce[4 * c:4 * c + nrow, :],
                                      stage[:nrow, :])
                    ynr = sup.tile([1, 512], F32, tag="ynrow")
                    nc.sync.dma_start(
                        ynr[0:1, :cw],
                        yn_bounce[4 * c:4 * c + nrow, :]
                        .rearrange("a b -> (a b)")
                        .rearrange("(o ab) -> o ab", o=1))
                    if c == NCHUNK - 1:
                        nc.vector.memset(ynr[0:1, NSHARD - 6144:cw], -NEGPAD)
                    nc.vector.tensor_copy(yn3[0:1, co:co + cw], ynr[0:1, :cw])
                    nc.vector.tensor_tensor(out=ynr[0:1, :cw],
                                            in0=ynr[0:1, :cw],
                                            in1=yn3[0:1, co:co + cw],
                                            op=AL.subtract)
                    st1 = sup.tile([1, 512], BF16, tag="ystage")
                    nc.vector.tensor_copy(st1[0:1, :cw], ynr[0:1, :cw])
                    nc.sync.dma_start(yn3[1:2, co:co + cw], st1[0:1, :cw])
                    nc.vector.tensor_tensor(out=ynr[0:1, :cw],
                                            in0=ynr[0:1, :cw],
                                            in1=st1[0:1, :cw], op=AL.subtract)
                    st2 = sup.tile([1, 512], BF16, tag="yresid")
                    nc.vector.tensor_copy(st2[0:1, :cw], ynr[0:1, :cw])
                    nc.sync.dma_start(yn3[2:3, co:co + cw], st2[0:1, :cw])

                # train side: 16 DMAs of 3 tiles + tail; fused casts
                nc.vector.memset(yn_nat[:, 48:50], 0.0)
                for k in range(4):
                    nc.vector.memset(
                        trh_all[:, k * COLS + NSHARD:(k + 1) * COLS]
                        .bitcast(F32), 0.0)
                for g in range(17):
                    nt = 3 if g < 16 else 1
                    rows = 128 if g < 16 else 106
                    tt = sup.tile([128, 3 * D], F32, tag="trload")
                    if g < 16:
                        nc.sync.dma_start(
                            tt[:].rearrange("p (j d) -> p j d", d=D),
                            tr_in[g * 384:(g + 1) * 384, :]
                            .rearrange("(j p) d -> p j d", p=128))
                    else:
                        nc.sync.dma_start(tt[:106, 0:D], tr_in[6144:6250, :])
                    for j in range(nt):
                        t = g * 3 + j
                        junk2 = sup.tile([128, D], F32, tag="junk")
                        nc.scalar.activation(junk2[:rows, :],
                                             tt[:rows, j * D:(j + 1) * D],
                                             AF.Square,
                                             accum_out=yn_nat[:rows, t:t + 1])
                        tp4 = aux_ps.tile([128, 512], F32, tag="tp")
                        for k in range(4):
                            nc.tensor.transpose(
                                tp4[:, k * 128:k * 128 + rows],
                                tt[:rows, j * D + k * 128:j * D + (k + 1) * 128],
                                ident[:rows, :rows])
                        cs = t * 128
                        dst = trh_v[:, :, cs:cs + rows]
                        src = tp4[:].rearrange("p (k n) -> p k n", k=4)[:, :, :rows]
                        nc.vector.tensor_copy(dst, src)
                        if t % 4 == 3:
                            ladder_chunk((t - 3) // 4)
                        elif t == 48:
                            ladder_chunk(12)
                    if g % 2 == 1 and g // 2 + 1 <= 7:
                        x_group(g // 2 + 1)

            # ------------- main + global phase -------------
            with tc.tile_pool(name="wmain", bufs=2) as wmain, \
                 tc.tile_pool(name="candp", bufs=3) as candp, \
                 tc.tile_pool(name="small", bufs=2) as small:

                pid_sp = nc.sync.partition_id()
                HB = B // 2

                gstate = {}

                def global_read_merge(l):
                    gv = small.tile([128, NG], F32, name=f"gv{l}", tag="gv")
                    gl = small.tile([128, NG], F32, name=f"gl{l}", tag="gl")
                    src = ag_out[l][:].rearrange("(c r) f -> r c f", c=NCORES)
                    nc.sync.dma_start(
                        gv[:].rearrange("p (c k) -> p c k", c=NCORES),
                        src[:, :, 0:K])
                    nc.sync.dma_start(
                        gl[:].rearrange("p (c k) -> p c k", c=NCORES),
                        src[:, :, K:2 * K])
                    nc.vector.tensor_tensor(out=gl[:], in0=gl[:],
                                            in1=coreoff_f[:], op=AL.add)
                    vlo = small.tile([128, NG], U16, name=f"gvlo{l}", tag="vlo")
                    vhi = small.tile([128, NG], U16, name=f"gvhi{l}", tag="vhi")
                    gvu = gv[:].bitcast(U16).rearrange("p (a two) -> p a two",
                                                       two=2)
                    nc.vector.tensor_copy(vlo[:], gvu[:, :, 0:1])
                    nc.vector.tensor_copy(vhi[:], gvu[:, :, 1:2])
                    glu = small.tile([128, NG], U16, name=f"glu{l}", tag="glu")
                    nc.vector.tensor_copy(glu[:], gl[:])
                    slo = small.tile([128, K], U16, name=f"gslo{l}",
                                     tag="slo16")
                    shi = small.tile([128, K], U16, name=f"gshi{l}",
                                     tag="shi16")
                    sla = small.tile([128, K], U16, name=f"gsla{l}",
                                     tag="sgi16")
                    sco = small.tile([128, K], U16, name=f"gsco{l}",
                                     tag="scoff")
                    _merge_top16(nc, small, f"gm{l}", gv, NG,
                                 [(vlo[:], slo), (vhi[:], shi), (glu[:], sla),
                                  (coff_row[:], sco)])
                    v16 = small.tile([128, K], F32, name=f"gv16{l}", tag="v16")
                    v16u = v16[:].bitcast(U16).rearrange(
                        "p (a two) -> p a two", two=2)
                    nc.vector.tensor_copy(v16u[:, :, 0:1], slo[:])
                    nc.vector.tensor_copy(v16u[:, :, 1:2], shi[:])
                    scof = small.tile([128, K], F32, name=f"scf{l}", tag="scof")
                    nc.vector.tensor_copy(scof[:], sco[:])
                    gidx = small.tile([128, K], F32, name=f"gix{l}",
                                      tag="gidx")
                    nc.vector.tensor_copy(gidx[:], sla[:])
                    gstate[l] = (v16, gidx, scof)

                def global_finish(l):
                    v16, gidx, scof = gstate[l]
                    cjunk = small.tile([128, NCORES * NCLASS], U16,
                                       name=f"cj{l}", tag="cjunk")
                    lab16 = small.tile([128, K], F32, name=f"glab{l}",
                                       tag="lab16")
                    for r in range(K):
                        nc.vector.tensor_scalar(
                            out=cjunk[:], in0=bndcat_u[:],
                            scalar1=gidx[:, r:r + 1], scalar2=None,
                            op0=AL.is_le, op1=AL.add,
                            accum_out=lab16[:, r:r + 1])
                    nc.vector.tensor_tensor(out=lab16[:], in0=lab16[:],
                                            in1=scof[:], op=AL.subtract)
                    nc.vector.tensor_scalar(out=lab16[:], in0=lab16[:],
                                            scalar1=-1.0, scalar2=None,
                                            op0=AL.add)
                    xn_col = small.tile([128, 1], F32, name=f"xnc{l}",
                                        tag="xncol")
                    nc.sync.dma_start(xn_col[:],
                                      xn_all[:, bass.ds(pid_sp + 8 * l, 1)])
                    dsq = small.tile([128, K], F32, name=f"dsq{l}", tag="dsq")
                    nc.scalar.activation(dsq[:], v16[:], AF.Sqrt, scale=-1.0,
                                         bias=xn_col[:, 0:1])
                    ew = small.tile([128, K], F32, name=f"ew{l}", tag="ew")
                    zsum = small.tile([128, 1], F32, name=f"zs{l}", tag="zs")
                    nc.scalar.activation(ew[:], dsq[:], AF.Exp, scale=-1.0,
                                         accum_out=zsum[:, 0:1])
                    rz = small.tile([128, 1], F32, name=f"rz{l}", tag="rz")
                    nc.vector.reciprocal(rz[:], zsum[:])
                    wt = small.tile([128, K], F32, name=f"wt{l}", tag="wt")
                    nc.vector.tensor_scalar(out=wt[:], in0=ew[:],
                                            scalar1=rz[:, 0:1], scalar2=None,
                                            op0=AL.mult)
                    vote = small.tile([128, NCLASS], F32, name=f"vote{l}",
                                      tag="vote")
                    tmp = small.tile([128, NCLASS], F32, name=f"vtmp{l}",
                                     tag="vtmp")
                    nc.vector.memset(vote[:], 0.0)
                    for r in range(K):
                        nc.vector.tensor_scalar(out=tmp[:], in0=cio_f[:],
                                                scalar1=lab16[:, r:r + 1],
                                                scalar2=wt[:, r:r + 1],
                                                op0=AL.is_equal, op1=AL.mult)
                        nc.vector.tensor_tensor(out=vote[:], in0=vote[:],
                                                in1=tmp[:], op=AL.add)
                    nc.sync.dma_start(out_d[l * 128:(l + 1) * 128, :], vote[:])

                for qt in range(QTILES if STAGE >= 2 else 0):
                    qs = qt * 128
                    cv = candp.tile([128, NCAND], F32, name=f"cv{qt}", tag="cv")
                    ci = candp.tile([128, NCAND], U16, name=f"ci{qt}", tag="ci")
                    for w, (c0, nsub, ww) in enumerate(WINDOWS):
                        zt = wmain.tile([128, 3328], F32, tag="zt")
                        for sub in range(nsub):
                            c = c0 + sub
                            cw = CHUNKS[c]
                            co = 512 * c
                            ps = zps.tile([128, 512], F32)
                            for k in range(4):
                                nc.tensor.matmul(
                                    ps[:, :cw],
                                    xh_v[:, k, qs:qs + 128],
                                    trh_all[:, k * COLS + co:
                                            k * COLS + co + cw],
                                    start=(k == 0), stop=False)
                            nc.tensor.matmul(ps[:, :cw], ones3[:],
                                             yn3[:, co:co + cw],
                                             start=False, stop=True)
                            nc.scalar.copy(zt[:, sub * 512:sub * 512 + cw],
                                           ps[:, :cw])
                        nc.vector.max(cv[:, w * 8:w * 8 + 8], zt[:, :ww])
                        nc.vector.max_index(ci[:, w * 8:w * 8 + 8],
                                            cv[:, w * 8:w * 8 + 8],
                                            zt[:, :ww])

                    # ship the 16 window-top-8 candidates unsorted; the
                    # global merge sorts by value anyway.
                    gi = small.tile([128, NCAND], U16, name=f"gi{qt}", tag="gi")
                    nc.vector.tensor_tensor(out=gi[:], in0=ci[:],
                                            in1=base16[:], op=AL.add)
                    gf = small.tile([128, K], F32, name=f"gf{qt}", tag="gf")
                    nc.vector.tensor_copy(gf[:], gi[:])
                    nc.sync.dma_start(ag_in[qt * 128:(qt + 1) * 128, 0:K],
                                      cv[:])
                    nc.sync.dma_start(
                        ag_in[qt * 128:(qt + 1) * 128, K:2 * K], gf[:])

                    if STAGE >= 3:
                        if qt in (7, 15):
                            h = qt // 8
                            nc.gpsimd.collective_compute(
                                "AllToAll", AL.bypass,
                                replica_groups=[list(range(NCORES))],
                                ins=[ag_in[h * 1024:(h + 1) * 1024, :].opt()],
                                outs=[ag_out[h][:].opt()])
                        if qt == 8:
                            global_read_merge(0)
                        elif qt == 10:
                            global_finish(0)
                        elif qt == 15:
                            global_read_merge(1)
                            global_finish(1)

    nc.finalize()
    return nc


_NC_CACHE = None


def kernel(x, train_features, train_labels, **run_kwargs):
    global _NC_CACHE
    x = np.ascontiguousarray(np.asarray(x, dtype=np.float32))
    tf = np.ascontiguousarray(np.asarray(train_features, dtype=np.float32))
    tl = np.asarray(train_labels)

    bnd_cat = np.zeros(NCORES * NCLASS, np.float32)
    shards = []
    for c in range(NCORES):
        sl = slice(c * NSHARD, (c + 1) * NSHARD)
        labs = np.asarray(tl[sl], dtype=np.int64)
        perm = np.argsort(labs, kind="stable")
        feats_s = np.ascontiguousarray(tf[sl][perm])
        labs_s = labs[perm]
        bnd = np.searchsorted(labs_s, np.arange(NCLASS), side="left")
        bnd_cat[c * NCLASS:(c + 1) * NCLASS] = c * CORESTRIDE + bnd
        shards.append(feats_s)

    in_maps = [{
        "x": x,
        "tr": shards[c],
        "bnd": bnd_cat[None, :],
    } for c in range(NCORES)]

    if _NC_CACHE is None:
        _NC_CACHE = build()
    res = bass_utils.run_bass_kernel_spmd(
        _NC_CACHE, in_maps, core_ids=list(range(NCORES)), **run_kwargs)
    global LAST_RESULTS
    LAST_RESULTS = res
    out = np.zeros((B, NCLASS), np.float32)
    for c in range(NCORES):
        oc = res.results[c]["out"]
        out[c * 128:(c + 1) * 128] = oc[0:128]
        out[1024 + c * 128:1024 + (c + 1) * 128] = oc[128:256]
    return out


LAST_RESULTS = None
